# revision 21
# baseline (speedup 1.0000x reference)
"""CardEncoder Trainium2 kernel.

Model (per sequence of L=16 tokens): embed(32) -> bidirectional LSTM(32) ->
concat final states -> per-batch dense (4096 -> 64) -> tanh.

Strategy (pure data parallel, 8 cores, batch-sharded):
  * Host packs a gather table [10112, 128] bf16 per vocab row:
      [ h_fw slot (zeros 0:32) | emb (32:64) | emb (64:96) | h_bw slot ]
  * Device gathers rows with NON-transpose dma_gather (row-major, one 256B
    packet per token) round-robin across 4 SWDGE queues -- the Q7
    descriptor-generation runs on a different core pair per queue, so up to
    4 gathers execute concurrently (transpose-mode gathers share the S2M
    xbar and corrupt each other when concurrent; row-major ones do not).
  * Each gathered tile R[128, 64, 128] is flipped to feature-major
    G[128, T*NT] by one HWDGE dma_start_transpose (xbar block transpose:
    out[p, j, f] = in[f, 128j + p]) -- zero compute-engine cost.
  * The constant-zero h-slots in G are overwritten in-place with the running
    hidden state so each LSTM step is ONE K=64 matmul per gate/stream.
  * 4 streams (2 seq-tiles x {fw, bw}) stacked on PSUM partition quarters via
    tile_position col-groups, so all elementwise work runs on full
    [128, NT] tiles.
  * LSTM cell: i,f,o = sigmoid, g = tanh (ACT); c = f*c + i*g (DVE, bf16);
    h = o*tanh(c) written by four quarter-width DVE muls straight into the
    next step's h-slots (no copies).
  * Dense head accumulated per-pair into a PSUM tile (overlapped with the
    LSTM instead of a serial tail); host transposes [64, B] -> [B, 64].
  * mask_zero=True handling: token==0 steps must leave (h, c) unchanged.
    Zero tokens occur w.p. 1e-4; the device ignores masking and the host
    recomputes the ~0.02% of affected sequences exactly and patches the
    affected output rows.
"""

import os
import numpy as np
import ml_dtypes

os.environ.setdefault("JAX_PLATFORMS", "cpu")

import concourse.bass as bass
import concourse.bacc as bacc
import concourse.mybir as mybir
import concourse.tile as tile
from concourse import bass_utils

BF16 = ml_dtypes.bfloat16

B, P, L = 2048, 64, 16
H = 32                      # LSTM units per direction
VOC = 10000
VOCP = 10112                # padded to 79 * 128
N_CORES = 8
B_LOC = B // N_CORES        # 256 batches per core
NSEQ = B_LOC * P            # 16384 sequences per core
NT = 512                    # sequences per tile
T = L
NQ = 4                      # SWDGE queues (concurrent gather core-pairs)

# G tile partition layout (SBUF APs must start at partition 0/32/64/96):
#   [ h_fw slot 0:32 | emb 32:64 | emb copy 64:96 | h_bw slot 96:128 ]
# fw rhs = rows 0:64 [h_fw, emb];  bw rhs = rows 64:128 [emb, h_bw].
HFW0 = 0
EMB0 = 32
EMB1 = 64
HBW0 = 96
KDIM = 64                   # matmul contraction size


def _f32(x):
    return np.asarray(x, np.float32)


# ---------------------------------------------------------------------------
# device kernel
# ---------------------------------------------------------------------------

def build_kernel(nseq=NSEQ, mode="full"):
    ntiles = nseq // NT
    npairs = ntiles // 2
    nbatch = nseq // P
    tiles_per_q = ntiles // NQ
    IDXW = NT * T // 16          # idx columns per tile
    BPP = 2 * NT // P            # batches per pair (16)

    nc = bacc.Bacc("TRN2", target_bir_lowering=False, debug=False,
                   enable_asserts=False, num_devices=N_CORES,
                   num_swdge_queues=NQ)

    table_d = nc.dram_tensor("table", [VOCP, 128], mybir.dt.bfloat16,
                             kind="ExternalInput")
    idx_d = nc.dram_tensor("idx", [128, tiles_per_q * IDXW], mybir.dt.int16,
                           kind="ExternalInput")
    wf_d = nc.dram_tensor("wf", [KDIM, 128], mybir.dt.bfloat16,
                          kind="ExternalInput")
    wb_d = nc.dram_tensor("wb", [128, 128], mybir.dt.bfloat16,
                          kind="ExternalInput")
    bv_d = nc.dram_tensor("bv", [128, 4], mybir.dt.float32,
                          kind="ExternalInput")
    wd_d = nc.dram_tensor("wd", [64, 4096], mybir.dt.bfloat16,
                          kind="ExternalInput")
    bd_d = nc.dram_tensor("bd", [64, 1], mybir.dt.float32,
                          kind="ExternalInput")
    out_d = nc.dram_tensor("out", [64, nbatch], mybir.dt.float32,
                           kind="ExternalOutput")
    state_d = nc.dram_tensor("state", [64, nseq], mybir.dt.bfloat16,
                             kind="ExternalOutput")

    FP32 = mybir.dt.float32
    BF = mybir.dt.bfloat16
    SIG = mybir.ActivationFunctionType.Sigmoid
    TANH = mybir.ActivationFunctionType.Tanh

    nsp = npairs // 2            # super-pairs: two pairs issue interleaved
    SHIFT = 3                    # pair B lags pair A by SHIFT steps

    with tile.TileContext(nc) as tc:
        with tc.tile_pool(name="const", bufs=1) as cpool:
          with tc.tile_pool(name="rbuf", bufs=4) as rpool, \
               tc.tile_pool(name="gbuf", bufs=8) as gpool, \
               tc.tile_pool(name="work", bufs=2) as wpool, \
               tc.tile_pool(name="stch", bufs=2) as spool, \
               tc.tile_pool(name="zps", bufs=1, space="PSUM") as zpool:

            wf = cpool.tile([KDIM, 128], BF)
            nc.sync.dma_start(out=wf[:, :], in_=wf_d.ap())
            # bw weights live at partition base 64: walrus requires matmul
            # fmap and weight to share the same SB start partition, and the
            # bw rhs is G[64:128]. Host pads to [128, 128] (top half zeros)
            # so the DMA itself writes at partition base 0.
            wb_t = cpool.tile([128, 128], BF)
            nc.sync.dma_start(out=wb_t[:, :], in_=wb_d.ap())
            wb = wb_t[64:128, :]
            bv = cpool.tile([128, 4], FP32)
            nc.sync.dma_start(out=bv[:, :], in_=bv_d.ap())
            idx_sb = cpool.tile([128, tiles_per_q * IDXW], mybir.dt.int16)
            nc.sync.dma_start(out=idx_sb[:, :], in_=idx_d.ap())
            wd = cpool.tile([64, 4096], BF)
            nc.sync.dma_start(out=wd[:, :], in_=wd_d.ap())
            bd = cpool.tile([64, 1], FP32)
            nc.sync.dma_start(out=bd[:, :], in_=bd_d.ap())

            out_sb = cpool.tile([64, nbatch], FP32)

            NIH = T * NT // 2        # idxs per half-tile gather (4096)
            IHW = NIH // 16          # idx columns per half (256)

            def gather_half(j, h):
                """Row-major gather of half h of tile j (4096 rows) on
                queue (2j+h)%NQ into a half-size R tile."""
                gh = 2 * j + h
                r = rpool.tile([128, NIH // 128, 128], BF, tag="r",
                               name=f"r{j}_{h}")
                nc.gpsimd.dma_gather(
                    out_ap=r[:, :, :],
                    in_ap=table_d.ap(),
                    idxs_ap=idx_sb[:, (gh // NQ) * IHW:(gh // NQ + 1) * IHW],
                    num_idxs=NIH,
                    num_idxs_reg=NIH,
                    elem_size=128,
                    transpose=False,
                    single_packet=False,
                    queue_num=gh % NQ,
                )
                return r

            def gather_batch(js):
                """Issue 4 half-gathers (one per SWDGE queue, concurrent),
                THEN their xbar block-transposes into the feature-major G
                tiles. The next batch's gathers WAR this batch's transposes
                (R reuse), giving a clean [4xgather | 4xtranspose]
                alternation on the DMA side."""
                if mode == "compute":
                    return {j: gpool.tile([128, 1, T * NT], BF, tag="g",
                                          name=f"g{j}") for j in js}
                rs = [(j, h, gather_half(j, h)) for j in js for h in (0, 1)]
                out = {}
                for j in js:
                    out[j] = gpool.tile([128, 1, T * NT], BF, tag="g",
                                        name=f"g{j}")
                for j, h, r in rs:
                    dst = out[j][:, 0, h * NIH:(h + 1) * NIH]
                    nc.sync.dma_start_transpose(
                        dst.rearrange("p (j f) -> p j f", f=128),
                        r[:, :, :])
                return out

            def phase_mm(ps, tau):
                """Gate matmuls for one step of one pair."""
                zb = (ps["pair"] % 2) * 4
                zt = [zpool.tile([128, NT], FP32, tag=f"z{zb + gi}",
                                 name=f"z{zb + gi}_{ps['pair']}_{tau}")
                      for gi in range(4)]
                for s in range(4):
                    g_t = ps["gs"][s // 2]
                    bw = s % 2
                    blk = (T - 1 - tau) if bw else tau
                    lo = EMB1 if bw else HFW0
                    w_s = wb if bw else wf
                    rhs = g_t[lo:lo + KDIM, 0, blk * NT:(blk + 1) * NT]
                    for gi in range(4):   # gate order i,f,g,o
                        nc.tensor.matmul(
                            zt[gi][32 * s:32 * s + 32, :],
                            w_s[:, 32 * gi:32 * gi + 32], rhs,
                            start=True, stop=True,
                            tile_position=(64 if bw else 0, 32 * s))
                ps["zt"] = zt

            def phase_acts(ps, tau):
                zt = ps["zt"]
                ui = wpool.tile([128, NT], BF, tag="ui")
                nc.scalar.activation(ui[:, :], zt[0][:, :], SIG,
                                     bias=bv[:, 0:1])
                uf = wpool.tile([128, NT], BF, tag="uf")
                nc.scalar.activation(uf[:, :], zt[1][:, :], SIG,
                                     bias=bv[:, 1:2])
                g_all = wpool.tile([128, NT], BF, tag="gall")
                nc.scalar.activation(g_all[:, :], zt[2][:, :], TANH,
                                     bias=bv[:, 2:3])
                uo = wpool.tile([128, NT], BF, tag="uo")
                nc.scalar.activation(uo[:, :], zt[3][:, :], SIG,
                                     bias=bv[:, 3:4])
                ps["u"] = (ui, uf, g_all, uo)

            def phase_cell(ps, tau):
                ui, uf, g_all, uo = ps["u"]
                c_all = ps["c"]
                if tau == 0:
                    # c = i * g  (h-slots and previous c are zero)
                    nc.vector.tensor_mul(c_all[:, :], ui[:, :], g_all[:, :])
                else:
                    # in-place products keep SBUF small: ui <- i*g, uf <- f*c
                    nc.vector.tensor_mul(ui[:, :], ui[:, :], g_all[:, :])
                    nc.vector.tensor_mul(uf[:, :], uf[:, :], c_all[:, :])
                    nc.vector.tensor_add(c_all[:, :], ui[:, :], uf[:, :])

            def phase_tc(ps, tau):
                tc_t = wpool.tile([128, NT], BF, tag="tc")
                nc.scalar.activation(tc_t[:, :], ps["c"][:, :], TANH)
                ps["tc"] = tc_t

            def phase_h(ps, tau):
                # h = o * tanh(c), written straight into each stream's
                # destination (next-step h-slot of G, or the state chunk
                # at the last step) -- no intermediate h tile / copies.
                uo = ps["u"][3]
                tc_t = ps["tc"]
                for s in range(4):
                    g_t = ps["gs"][s // 2]
                    bw = s % 2
                    if tau == T - 1:
                        dst = ps["st"][32 * bw:32 * bw + 32,
                                       (s // 2) * NT:(s // 2 + 1) * NT]
                    else:
                        nblk = (T - 2 - tau) if bw else (tau + 1)
                        h0 = HBW0 if bw else HFW0
                        dst = g_t[h0:h0 + 32, 0,
                                  nblk * NT:(nblk + 1) * NT]
                    nc.vector.tensor_mul(dst,
                                         uo[32 * s:32 * s + 32, :],
                                         tc_t[32 * s:32 * s + 32, :])

            def issue_head(ps):
                """Dense head for this pair's BPP batches: 64 accumulating
                matmuls into a reused z-bank, tanh straight to out_sb, and
                the state chunk DMA (on the ACT hwdge queue so the SP queue
                stays clear for the JIT transposes)."""
                pair, st = ps["pair"], ps["st"]
                zh = zpool.tile([128, NT], FP32, tag=f"z{(pair % 2) * 4}",
                                name=f"zh{pair}")
                st_r = st[:, :].rearrange("c (b q) -> c q b", q=P)
                for p in range(P):
                    nc.tensor.matmul(
                        zh[0:64, 0:BPP],
                        wd[:, 64 * p:64 * p + 64], st_r[:, p:p + 1, :],
                        start=(p == 0), stop=(p == P - 1))
                nc.scalar.activation(out_sb[:, pair * BPP:(pair + 1) * BPP],
                                     zh[0:64, 0:BPP], TANH, bias=bd[:, :])
                nc.scalar.dma_start(
                    out=state_d.ap()[:, pair * 2 * NT:(pair + 1) * 2 * NT],
                    in_=st[:, :])

            # ---- rolling software pipeline: pair p starts HALF steps after
            # pair p-1, so every wave issues one step of each of two pairs,
            # phase-interleaved, and each in-order engine queue always has
            # work from the other chain to hide the current chain's latency.
            HALF = T // 2
            BATCH = 2                 # tiles (= 4 half-gathers) per DMA batch
            tiles = {}
            next_tile = [0]

            def issue_batch():
                js = list(range(next_tile[0],
                                min(next_tile[0] + BATCH, ntiles)))
                if js:
                    tiles.update(gather_batch(js))
                    next_tile[0] = js[-1] + 1

            def start_pair(p):
                return {"pair": p,
                        "gs": [tiles.pop(2 * p), tiles.pop(2 * p + 1)],
                        "st": spool.tile([64, 2 * NT], BF, tag="st",
                                         name=f"st{p}"),
                        "c": wpool.tile([128, NT], BF, tag="c",
                                        name=f"c{p}")}

            if mode == "gather":
                while next_tile[0] < ntiles or tiles:
                    if next_tile[0] < ntiles:
                        issue_batch()
                    for j in sorted(tiles):
                        if j % 2 == 1 and (j - 1) in tiles:
                            pair = j // 2
                            st = spool.tile([64, 2 * NT], BF, tag="st")
                            for gi2 in range(2):
                                nc.vector.tensor_copy(
                                    st[0:32, gi2 * NT:(gi2 + 1) * NT],
                                    tiles[j - 1 + gi2][32:64, 0,
                                                       (T - 1) * NT:T * NT])
                            nc.scalar.dma_start(
                                out=state_d.ap()[:, pair * 2 * NT:
                                                 (pair + 1) * 2 * NT],
                                in_=st[:, :])
                            tiles.pop(j - 1)
                            tiles.pop(j)
            elif mode != "empty":
                issue_batch()
                issue_batch()             # prologue: 4 tiles in flight
                active = []
                total_waves = (npairs - 1) * HALF + T
                for w in range(total_waves):
                    if w % HALF == 0 and w // HALF < npairs:
                        active.append(start_pair(w // HALF))
                        issue_batch()     # prefetch pair p+2's tiles
                    # (pair-state, tau) for this wave, oldest pair first
                    cur = [(ps, w - ps["pair"] * HALF) for ps in active]
                    cur = [(ps, tau) for ps, tau in cur if 0 <= tau < T]
                    for ph in (phase_mm, phase_acts, phase_cell, phase_tc,
                               phase_h):
                        for ps, tau in cur:
                            ph(ps, tau)
                    for ps, tau in cur:
                        if tau == T - 1:
                            issue_head(ps)
                            active.remove(ps)

            if mode == "full":
                nc.sync.dma_start(out=out_d.ap(), in_=out_sb[:, :])

    nc.compile()
    return nc


# ---------------------------------------------------------------------------
# host-side packing
# ---------------------------------------------------------------------------

def pack_table(embed_table):
    tbl = np.zeros((VOCP, 128), np.float32)
    tbl[:VOC, EMB0:EMB0 + 32] = _f32(embed_table)
    tbl[:VOC, EMB1:EMB1 + 32] = _f32(embed_table)
    return tbl.astype(BF16)


def pack_idx(x_core, nseq=NSEQ):
    """x_core: [nseq, T] int32 -> int16 [128, nhalves//NQ * IHW], banded:
    half-tile gh=2j+h lives in partitions [32*(gh%NQ), +32) (the 16-row
    wrap duplicated twice: the queue's rx and tx Q7 cores each read their
    own 16 partitions), at column block gh//NQ."""
    ntiles = nseq // NT
    NIH = NT * T // 2
    IHW = NIH // 16
    arr = np.zeros((128, (2 * ntiles // NQ) * IHW), np.int16)
    for j in range(ntiles):
        u = x_core[j * NT:(j + 1) * NT, :].T.reshape(-1)      # t-major [T*NT]
        for h in (0, 1):
            gh = 2 * j + h
            q, k = gh % NQ, gh // NQ
            w = u[h * NIH:(h + 1) * NIH].reshape(-1, 16).T     # [16, IHW]
            arr[32 * q:32 * q + 32, k * IHW:(k + 1) * IHW] = np.tile(w, (2, 1))
    return arr


def pack_weights(Wk, Wr, b):
    Wk, Wr, b = _f32(Wk), _f32(Wr), _f32(b)
    wf = np.concatenate([Wr, Wk], 0)                          # [64, 128]
    wb = np.concatenate([np.zeros((64, 128), np.float32), Wk, Wr], 0)
    bv = np.tile(b.reshape(4, 32), (1, 4)).reshape(4, 128).T  # [128, 4]
    return wf.astype(BF16), wb.astype(BF16), np.ascontiguousarray(bv, np.float32)


def pack_wd(Wd):
    w = _f32(Wd).reshape(P, 64, 64).transpose(1, 0, 2).reshape(64, 4096)
    return w.astype(BF16)


# ---------------------------------------------------------------------------
# host reference bits for the zero-token fixup
# ---------------------------------------------------------------------------

def _np_lstm_last_h(emb, mask, Wk, Wr, b):
    n = emb.shape[0]
    h = np.zeros((n, H), np.float32)
    c = np.zeros((n, H), np.float32)
    for t in range(emb.shape[1]):
        z = emb[:, t, :] @ Wk + h @ Wr + b
        i = 1.0 / (1.0 + np.exp(-z[:, 0:32]))
        f = 1.0 / (1.0 + np.exp(-z[:, 32:64]))
        g = np.tanh(z[:, 64:96])
        o = 1.0 / (1.0 + np.exp(-z[:, 96:128]))
        c_new = f * c + i * g
        h_new = o * np.tanh(c_new)
        m = mask[:, t][:, None]
        h = np.where(m, h_new, h)
        c = np.where(m, c_new, c)
    return h


def _host_fixup(out, state_all, x_flat, embed_table, Wk, Wr, b, Wd, bd):
    """Recompute rows whose sequences contain a zero token."""
    mask = x_flat != 0
    bad_seq = np.nonzero(~mask.all(axis=1))[0]
    if bad_seq.size == 0:
        return out
    emb = _f32(embed_table)[x_flat[bad_seq]]
    h_fw = _np_lstm_last_h(emb, mask[bad_seq], _f32(Wk), _f32(Wr), _f32(b))
    h_bw = _np_lstm_last_h(emb[:, ::-1, :], mask[bad_seq][:, ::-1],
                           _f32(Wk), _f32(Wr), _f32(b))
    state_all = state_all.copy()
    state_all[bad_seq] = np.concatenate([h_fw, h_bw], axis=1)
    bad_rows = np.unique(bad_seq // P)
    st = state_all[bad_rows[:, None] * P + np.arange(P)[None, :]]
    st = st.reshape(bad_rows.size, P * 64)
    out[bad_rows] = np.tanh(st @ _f32(Wd) + _f32(bd))
    return out


# ---------------------------------------------------------------------------
# entry point
# ---------------------------------------------------------------------------

_NC_CACHE = {}


def _get_nc(mode="full"):
    key = "nc" + mode
    if key not in _NC_CACHE:
        _NC_CACHE[key] = build_kernel(mode=mode)
    return _NC_CACHE[key]


def run_device(inputs, trace=False):
    x = np.asarray(inputs["x"])
    table = pack_table(inputs["embed_table"])
    wf, wb, bv = pack_weights(inputs["Wk"], inputs["Wr"], inputs["b"])
    wd = pack_wd(inputs["Wd"])
    bd = _f32(inputs["bd"]).reshape(64, 1)

    x_flat = x.reshape(B * P, L)
    in_maps = []
    for k in range(N_CORES):
        x_core = x_flat[k * NSEQ:(k + 1) * NSEQ]
        in_maps.append({
            "table": table,
            "idx": pack_idx(x_core),
            "wf": wf,
            "wb": wb,
            "bv": bv,
            "wd": wd,
            "bd": bd,
        })

    nc = _get_nc()
    res = bass_utils.run_bass_kernel_spmd(
        nc, in_maps, core_ids=list(range(N_CORES)), trace=trace)

    out = np.empty((B, 64), np.float32)
    state_all = np.empty((B * P, 64), np.float32)
    for k in range(N_CORES):
        out[k * B_LOC:(k + 1) * B_LOC] = res.results[k]["out"].T
        state_all[k * NSEQ:(k + 1) * NSEQ] = \
            _f32(res.results[k]["state"]).T
    return out, state_all, res


def kernel(x, embed_table, Wk, Wr, b, Wd, bd):
    inputs = dict(x=x, embed_table=embed_table, Wk=Wk, Wr=Wr, b=b,
                  Wd=Wd, bd=bd)
    out, state_all, _ = run_device(inputs)
    out = _host_fixup(out, state_all, np.asarray(x).reshape(B * P, L),
                      embed_table, Wk, Wr, b, Wd, bd)
    return out
